# revision 13
# baseline (speedup 1.0000x reference)
"""AdaptiveLaplacian TRN2 kernel (8 NeuronCores, SPMD).

reference math:
  v   = u @ W1                  (b1 cancels in the neighbor difference)
  lap = mean_j relu(v[idx[:,j]] - v)
  h   = lap @ W2                (b2 cancels in BatchNorm)
  out = relu((h - mean)*rsqrt(var+eps)*gamma + beta)   (stats over all N)

Strategy (points sharded 8 ways, weights replicated):
  P0  every core computes the FULL v table (fp16) into a 5-stripe DRAM
      layout: 32767 real rows + 1 dummy(-60000) row per stripe, so each
      stripe is addressable by int16 dma_gather indices.
  P1  per 128-point tile (partition p = point p): multi-queue dma_gather
      pulls all neighbor rows (window-sorted + padded per a host-built
      static schedule; dummies land on the -60000 row);
      in-place DVE max(x, ctr)  [max(x,c) = relu(x-c)+c, dummy -> c];
      j-sum via identity-matmul PSUM accumulation over B(t) blocks;
      lap = psum/16 - (B(t)/16)*ctr        (B(t) is compile-time);
      TensorE transpose; hT = W2.T @ lapT  -> channels on partitions;
      per-channel BN partial sums; hT stored to DRAM (fp16).
  P2  AllReduce the [128,2] BN stats over the 8 cores; scale/shift.
  P3  hT chunks: scale*h+shift, relu, DMA out (u_tt.T per core).

Host: per-core point permutation groups points with similar per-window
neighbor-count profiles into tiles (gather padding ~2.2x -> ~1.3x); the
call schedule is the max over cores so the SPMD graph is identical.
"""

import sys

sys.path.insert(0, "/opt/trn_rl_repo")

import numpy as np

import concourse.bass as bass
import concourse.mybir as mybir
import concourse.tile as tile
from concourse import bacc

F32 = mybir.dt.float32
FP16 = mybir.dt.float16
I16 = mybir.dt.int16

NCORES = 8
C = 128
NS = 16
BN_EPS = 1e-5
DUMMY_VAL = -60000.0
MAXBLK = 8  # dma_gather num_idxs <= 1024  ->  <= 8 blocks of 128 rows


def _dims(N=131072, STRIPE=32767):
    d = {}
    d["N"] = N
    d["STRIPE"] = STRIPE
    d["NSTRIPES"] = (N + STRIPE - 1) // STRIPE
    d["SROWS"] = STRIPE + 1
    d["SH"] = N // NCORES
    d["TILES"] = d["SH"] // 128
    d["CH"] = 8  # P0 point-tiles per chunk
    return d


DIMS = _dims()


def _host_prepare(d, u, idx, W1, W2, gamma, beta):
    N, SH, TILES = d["N"], d["SH"], d["TILES"]
    STRIPE, NSTRIPES = d["STRIPE"], d["NSTRIPES"]
    win = (idx // STRIPE).astype(np.int64)
    loc = (idx - win * STRIPE).astype(np.int64)

    perms, counts = [], []
    for k in range(NCORES):
        w_k = win[k * SH : (k + 1) * SH]
        prof = np.stack([(w_k == q).sum(1) for q in range(NSTRIPES)], 1)
        key = np.zeros(SH, np.int64)
        for q in range(NSTRIPES - 1):
            key = key * (NS + 1) + prof[:, q]
        perm = np.argsort(key, kind="stable")
        perms.append(perm)
        counts.append(prof[perm])

    B = np.zeros((TILES, NSTRIPES), np.int64)
    for k in range(NCORES):
        c = counts[k].reshape(TILES, 128, NSTRIPES)
        B = np.maximum(B, c.max(axis=1))

    callspec = []
    for t in range(TILES):
        calls, off = [], 0
        for w in range(NSTRIPES):
            rem = int(B[t, w])
            while rem > 0:
                nb = min(rem, MAXBLK)
                calls.append((w, nb, off))
                off += nb
                rem -= nb
        callspec.append((calls, off))

    idxcols = sum(8 * nb for calls, _ in callspec for (_, nb, _) in calls)

    idx_inputs = []
    for k in range(NCORES):
        perm = perms[k]
        w_k = win[k * SH : (k + 1) * SH][perm]
        l_k = loc[k * SH : (k + 1) * SH][perm]
        arr = np.full((128, idxcols), STRIPE, np.int16)  # default dummy
        col = 0
        for t in range(TILES):
            calls, _ = callspec[t]
            wt = w_k[t * 128 : (t + 1) * 128]
            lt = l_k[t * 128 : (t + 1) * 128]
            per_w = [
                [lt[p][wt[p] == w] for w in range(NSTRIPES)] for p in range(128)
            ]
            consumed = [[0] * NSTRIPES for _ in range(128)]
            for (w, nb, _off) in calls:
                ncols = 8 * nb
                sub = np.full((16, ncols), STRIPE, np.int16)
                for p in range(128):
                    lst = per_w[p][w]
                    c0 = consumed[p][w]
                    take = lst[c0 : c0 + nb]
                    consumed[p][w] = c0 + len(take)
                    for j, v in enumerate(take):
                        k_ord = j * 128 + p
                        sub[k_ord % 16, k_ord // 16] = v
                arr[0:16, col : col + ncols] = sub
                col += ncols
        for g in range(1, 8):
            arr[g * 16 : (g + 1) * 16] = arr[0:16]
        idx_inputs.append(arr)

    uT16 = np.ascontiguousarray(u.T.astype(np.float16))
    in_maps = []
    for k in range(NCORES):
        in_maps.append(
            {
                "uT": uT16,
                "uTo": np.ascontiguousarray(
                    u[k * SH : (k + 1) * SH][perms[k]].T.astype(np.float16)
                ),
                "idxg": idx_inputs[k],
                "W1": W1.astype(np.float16),
                "W2": W2.astype(np.float16),
                "ident": (np.eye(128) / 16.0).astype(np.float16),
                "gamma": gamma.astype(np.float32).reshape(C, 1),
                "beta": beta.astype(np.float32).reshape(C, 1),
            }
        )
    return in_maps, perms, callspec, idxcols


def _build(d, callspec, idxcols):
    N, SH, TILES, CH = d["N"], d["SH"], d["TILES"], d["CH"]
    STRIPE, NSTRIPES, SROWS = d["STRIPE"], d["NSTRIPES"], d["SROWS"]
    DUMMY_LOCAL = STRIPE

    nc = bacc.Bacc(target_bir_lowering=False, debug=False, num_swdge_queues=4)

    uT_d = nc.declare_dram_parameter("uT", [C, N], FP16, isOutput=False)
    uTo_d = nc.declare_dram_parameter("uTo", [C, SH], FP16, isOutput=False)
    idx_d = nc.declare_dram_parameter("idxg", [128, idxcols], I16, isOutput=False)
    W1_d = nc.declare_dram_parameter("W1", [C, C], FP16, isOutput=False)
    W2_d = nc.declare_dram_parameter("W2", [C, C], FP16, isOutput=False)
    id_d = nc.declare_dram_parameter("ident", [128, 128], FP16, isOutput=False)
    ga_d = nc.declare_dram_parameter("gamma", [C, 1], F32, isOutput=False)
    be_d = nc.declare_dram_parameter("beta", [C, 1], F32, isOutput=False)
    out_d = nc.declare_dram_parameter("out", [C, SH], F32, isOutput=True)

    v5 = nc.dram_tensor("v5", [NSTRIPES * SROWS, C], FP16)
    h_dram = nc.dram_tensor("h_dram", [C, SH], FP16)
    ccin = nc.dram_tensor("ccin", [128, 2], F32)
    ccout = nc.dram_tensor("ccout", [128, 2], F32, addr_space="Shared")

    Bmax = max(off for _, off in callspec)
    wbmax = {}
    for calls, _ in callspec:
        per_w = {}
        for (w, nb, _o) in calls:
            per_w[w] = per_w.get(w, 0) + nb
        for w, b in per_w.items():
            wbmax[w] = max(wbmax.get(w, 0), b)

    boundaries = [s * STRIPE for s in range(1, NSTRIPES)]

    with tile.TileContext(nc) as tc:
        with (
            tc.tile_pool(name="const", bufs=1) as cp,
            tc.tile_pool(name="ld", bufs=3) as lp,
            tc.tile_pool(name="work", bufs=2) as wp,
            tc.tile_pool(name="big", bufs=2) as bp,
            tc.tile_pool(name="p0", bufs=3) as p0p,
            tc.tile_pool(name="ps", bufs=2, space="PSUM") as pp,
        ):
            W1 = cp.tile([C, C], FP16, tag="W1")
            nc.sync.dma_start(out=W1[:], in_=W1_d[:, :])
            W2 = cp.tile([C, C], FP16, tag="W2")
            nc.sync.dma_start(out=W2[:], in_=W2_d[:, :])
            ident = cp.tile([128, 128], FP16, tag="ident")
            nc.sync.dma_start(out=ident[:], in_=id_d[:, :])
            stats = cp.tile([128, 2 * TILES], F32, tag="stats")
            bn = cp.tile([128, 16], F32, tag="bn")
            nc.sync.dma_start(out=bn[:, 8:9], in_=ga_d[:, :])
            nc.sync.dma_start(out=bn[:, 9:10], in_=be_d[:, :])

            idxg = cp.tile([128, idxcols], I16, tag="idxg")
            nc.sync.dma_start(out=idxg[:], in_=idx_d[:, :])
            dummy = cp.tile([1, C], FP16, tag="dummy")
            nc.vector.memset(dummy[:], DUMMY_VAL)
            for s in range(NSTRIPES):
                nc.sync.dma_start(
                    out=v5[s * SROWS + DUMMY_LOCAL : s * SROWS + DUMMY_LOCAL + 1, :],
                    in_=dummy[:],
                )

            # ---------------- P0: v table ----------------
            def emit_rows(vt, g0, nrows):
                """DMA vt ([128, CH*C], tile i col-block = rows g0+128i..) to v5."""
                g1 = g0 + nrows
                straddle = any(g0 < b < g1 for b in boundaries)
                if not straddle:
                    s = g0 // STRIPE
                    dst = s * SROWS + (g0 - s * STRIPE)
                    nc.sync.dma_start(
                        out=v5[dst : dst + nrows, :].rearrange(
                            "(i p) c -> p i c", p=128
                        ),
                        in_=vt[:, : (nrows // 128) * C].rearrange(
                            "p (i c) -> p i c", c=C
                        ),
                    )
                    return
                for i in range(nrows // 128):
                    r0 = g0 + i * 128
                    bs = [b for b in boundaries if r0 < b < r0 + 128]
                    cuts = [r0] + bs + [r0 + 128]
                    for a, b_ in zip(cuts[:-1], cuts[1:]):
                        s = a // STRIPE
                        dst = s * SROWS + (a - s * STRIPE)
                        nc.sync.dma_start(
                            out=v5[dst : dst + (b_ - a), :],
                            in_=vt[a - r0 : b_ - r0, i * C : (i + 1) * C],
                        )

            for ch in range(N // (128 * CH)):
                ut = p0p.tile([C, 128 * CH], FP16, tag="ut")
                nc.sync.dma_start(
                    out=ut[:], in_=uT_d[:, ch * 128 * CH : (ch + 1) * 128 * CH]
                )
                vt = p0p.tile([128, CH * C], FP16, tag="vt")
                for i in range(CH):
                    ps = pp.tile([128, C], F32, tag="acc", space="PSUM")
                    nc.tensor.matmul(
                        out=ps[:],
                        lhsT=ut[:, i * 128 : (i + 1) * 128],
                        rhs=W1[:],
                        start=True,
                        stop=True,
                    )
                    if i % 2 == 0:
                        nc.scalar.mul(vt[:, i * C : (i + 1) * C], ps[:], 1.0)
                    else:
                        nc.vector.tensor_copy(
                            out=vt[:, i * C : (i + 1) * C], in_=ps[:]
                        )
                emit_rows(vt, ch * 128 * CH, 128 * CH)

            # ---------------- P1 ----------------
            # pre-loaded num_idxs registers (one per distinct value) so the
            # gather calls carry no per-call register MOVE (a MOVE to a
            # register read by an in-flight gather serializes the queue)
            nb_vals = sorted({nb for calls, _ in callspec for (_, nb, _) in calls})
            nidx_regs = {nb: nc.gpsimd.to_reg(128 * nb) for nb in nb_vals}
            gsem = nc.alloc_semaphore("gsem")
            cum_calls = [0]
            for calls, _ in callspec:
                cum_calls.append(cum_calls[-1] + len(calls))
            qctr = 0
            col = 0
            for t in range(TILES):
                calls, Bt = callspec[t]
                ncols_t = sum(8 * nb for (_, nb, _) in calls)

                uot = lp.tile([C, 128], FP16, tag="uot")
                nc.sync.dma_start(
                    out=uot[:], in_=uTo_d[:, t * 128 : (t + 1) * 128]
                )

                psc = pp.tile([128, C], F32, tag="psc", space="PSUM")
                nc.tensor.matmul(
                    out=psc[:], lhsT=uot[:], rhs=W1[:], start=True, stop=True
                )
                ctr = wp.tile([128, C], FP16, tag="ctr")
                nc.scalar.mul(ctr[:], psc[:], 1.0)

                # per-window gather tiles: separate dep units so the 4
                # SWDGE queues generate descriptors concurrently
                wblocks = {}
                for (w, nb, boff) in calls:
                    wblocks[w] = wblocks.get(w, 0) + nb
                gtiles = {}
                mtiles = {}
                for (w, nb, boff) in calls:
                    if w not in gtiles:
                        gtiles[w] = (
                            bp.tile(
                                [128, wbmax[w] * C], FP16, tag=f"g{w}",
                                name=f"g{w}_t{t}",
                            ),
                            bp.tile(
                                [128, wbmax[w] * C], FP16, tag=f"m{w}",
                                name=f"m{w}_t{t}",
                            ),
                        )
                ccol = 0
                with tc.tile_critical():
                    if t >= 2:
                        nc.gpsimd.wait_ge(gsem, 16 * cum_calls[t - 1])
                    for (w, nb, boff) in calls:
                        gw = gtiles[w][0]
                        woff = sum(
                            b2 for (w2, b2, o2) in calls if w2 == w and o2 < boff
                        )
                        nc.gpsimd.dma_gather(
                            gw[:, woff * C : (woff + nb) * C].rearrange(
                                "p (b e) -> p b e", e=C
                            ),
                            v5[w * SROWS : (w + 1) * SROWS, :],
                            idxg[:, col + ccol : col + ccol + 8 * nb],
                            128 * nb,
                            nidx_regs[nb],
                            C,
                            queue_num=qctr % 4,
                        ).then_inc(gsem, 16)
                        qctr += 1
                        ccol += 8 * nb
                col += ncols_t

                gate = 16 * cum_calls[t + 1]
                for w, (gw, mw) in gtiles.items():
                    bw = wblocks[w]
                    with tc.tile_critical():
                        nc.vector.tensor_tensor(
                            out=mw[:, : bw * C].rearrange("p (b c) -> p b c", c=C),
                            in0=gw[:, : bw * C].rearrange("p (b c) -> p b c", c=C),
                            in1=ctr[:]
                            .rearrange("p (o c) -> p o c", o=1)
                            .to_broadcast([128, bw, C]),
                            op=mybir.AluOpType.max,
                        )._wait_ge(gsem, gate)

                psl = pp.tile([128, C], F32, tag="acc", space="PSUM")
                nmm = sum(wblocks.values())
                bi = 0
                for w in sorted(gtiles):
                    mw = gtiles[w][1]
                    for b in range(wblocks[w]):
                        nc.tensor.matmul(
                            out=psl[:],
                            lhsT=ident[:],
                            rhs=mw[:, b * C : (b + 1) * C],
                            start=(bi == 0),
                            stop=(bi == nmm - 1),
                        )
                        bi += 1

                ctrs = wp.tile([128, C], FP16, tag="ctrs")
                nc.scalar.mul(ctrs[:], psc[:], float(Bt) / 16.0)
                lap = wp.tile([128, C], FP16, tag="lap")
                nc.vector.tensor_tensor(
                    out=lap[:], in0=psl[:], in1=ctrs[:],
                    op=mybir.AluOpType.subtract,
                )

                pst = pp.tile([128, C], FP16, tag="pst", space="PSUM")
                nc.tensor.transpose(out=pst[:], in_=lap[:], identity=ident[:])
                lapT = wp.tile([128, C], FP16, tag="lapT")
                nc.scalar.mul(lapT[:], pst[:], 1.0)

                psh = pp.tile([128, C], F32, tag="psh", space="PSUM")
                nc.tensor.matmul(
                    out=psh[:], lhsT=W2[:], rhs=lapT[:], start=True, stop=True
                )
                h16 = wp.tile([128, C], FP16, tag="h16")
                nc.scalar.mul(h16[:], psh[:], 1.0)
                nc.sync.dma_start(
                    out=h_dram[:, t * 128 : (t + 1) * 128], in_=h16[:]
                )

                nc.vector.reduce_sum(
                    out=stats[:, t : t + 1], in_=psh[:], axis=mybir.AxisListType.X
                )
                sq = wp.tile([128, C], F32, tag="sq")
                nc.vector.tensor_tensor(
                    out=sq[:], in0=h16[:], in1=h16[:], op=mybir.AluOpType.mult
                )
                nc.vector.reduce_sum(
                    out=stats[:, TILES + t : TILES + t + 1],
                    in_=sq[:],
                    axis=mybir.AxisListType.X,
                )

            # ---------------- P2: BN stats ----------------
            nc.vector.reduce_sum(
                out=bn[:, 0:1], in_=stats[:, :TILES], axis=mybir.AxisListType.X
            )
            nc.vector.reduce_sum(
                out=bn[:, 1:2], in_=stats[:, TILES:], axis=mybir.AxisListType.X
            )
            cc_sem = nc.alloc_semaphore("cc_sem")
            dma_sem = nc.alloc_semaphore("cc_dma")
            with tc.tile_critical():
                nc.gpsimd.dma_start(out=ccin[:, :], in_=bn[:, 0:2]).then_inc(
                    dma_sem, 16
                )
                nc.gpsimd.wait_ge(dma_sem, 16)
                nc.gpsimd.collective_compute(
                    "AllReduce",
                    mybir.AluOpType.add,
                    replica_groups=[list(range(NCORES))],
                    ins=[ccin[:, :]],
                    outs=[ccout[:, :]],
                ).then_inc(cc_sem, 1)
                nc.gpsimd.wait_ge(cc_sem, 1)
                nc.gpsimd.dma_start(out=bn[:, 2:4], in_=ccout[:, :]).then_inc(
                    dma_sem, 16
                )
                nc.gpsimd.wait_ge(dma_sem, 32)

            nc.vector.tensor_scalar(
                out=bn[:, 4:5], in0=bn[:, 2:3], scalar1=1.0 / N, scalar2=None,
                op0=mybir.AluOpType.mult,
            )  # mean
            nc.vector.tensor_scalar(
                out=bn[:, 5:6], in0=bn[:, 3:4], scalar1=1.0 / N, scalar2=None,
                op0=mybir.AluOpType.mult,
            )  # E[x^2]
            nc.vector.tensor_tensor(
                out=bn[:, 6:7], in0=bn[:, 4:5], in1=bn[:, 4:5],
                op=mybir.AluOpType.mult,
            )
            nc.vector.tensor_tensor(
                out=bn[:, 7:8], in0=bn[:, 5:6], in1=bn[:, 6:7],
                op=mybir.AluOpType.subtract,
            )  # var
            nc.vector.tensor_scalar(
                out=bn[:, 10:11], in0=bn[:, 7:8], scalar1=BN_EPS, scalar2=None,
                op0=mybir.AluOpType.add,
            )
            nc.vector.reciprocal(out=bn[:, 15:16], in_=bn[:, 10:11])
            nc.scalar.activation(
                bn[:, 11:12], bn[:, 15:16], mybir.ActivationFunctionType.Sqrt
            )
            nc.vector.tensor_tensor(
                out=bn[:, 12:13], in0=bn[:, 8:9], in1=bn[:, 11:12],
                op=mybir.AluOpType.mult,
            )  # scale
            nc.vector.tensor_tensor(
                out=bn[:, 13:14], in0=bn[:, 4:5], in1=bn[:, 12:13],
                op=mybir.AluOpType.mult,
            )
            nc.vector.tensor_tensor(
                out=bn[:, 14:15], in0=bn[:, 9:10], in1=bn[:, 13:14],
                op=mybir.AluOpType.subtract,
            )  # shift

            # ---------------- P3 ----------------
            PC = 8
            for ch in range(TILES // PC):
                hch = wp.tile([128, 128 * PC], FP16, tag="hch")
                nc.sync.dma_start(
                    out=hch[:],
                    in_=h_dram[:, ch * 128 * PC : (ch + 1) * 128 * PC],
                )
                o2 = wp.tile([128, 128 * PC], F32, tag="o2")
                nc.scalar.activation(
                    o2[:],
                    hch[:],
                    mybir.ActivationFunctionType.Relu,
                    bias=bn[:, 14:15],
                    scale=bn[:, 12:13],
                )
                nc.sync.dma_start(
                    out=out_d[:, ch * 128 * PC : (ch + 1) * 128 * PC], in_=o2[:]
                )

    nc.compile()
    return nc


def _run(d, u, idx, W1, W2, gamma, beta, run_kwargs=None):
    from concourse.bass_utils import run_bass_kernel_spmd

    in_maps, perms, callspec, idxcols = _host_prepare(
        d, u, idx, W1, W2, gamma, beta
    )
    nc = _build(d, callspec, idxcols)
    res = run_bass_kernel_spmd(
        nc, in_maps, core_ids=list(range(NCORES)), **(run_kwargs or {})
    )
    SH, N = d["SH"], d["N"]
    u_tt = np.empty((N, C), np.float32)
    for k in range(NCORES):
        shard = res.results[k]["out"].T
        inv = np.empty(SH, np.int64)
        inv[perms[k]] = np.arange(SH)
        u_tt[k * SH : (k + 1) * SH] = shard[inv]
    return u_tt, res


def kernel(p, u, o, idx, W1, b1, W2, b2, gamma, beta):
    u_tt, _ = _run(
        DIMS,
        np.asarray(u, np.float32),
        np.asarray(idx, np.int64),
        np.asarray(W1),
        np.asarray(W2),
        np.asarray(gamma),
        np.asarray(beta),
    )
    return (
        np.asarray(p, np.float32),
        u_tt,
        np.asarray(o, np.int32),
    )


# revision 18
# speedup vs baseline: 2.0445x; 2.0445x over previous
"""AdaptiveLaplacian TRN2 kernel (8 NeuronCores, SPMD).

reference math:
  v   = u @ W1                  (b1 cancels in the neighbor difference)
  lap = mean_j relu(v[idx[:,j]] - v)
  h   = lap @ W2                (b2 cancels in BatchNorm)
  out = relu((h - mean)*rsqrt(var+eps)*gamma + beta)   (stats over all N)

Strategy (points sharded 8 ways, weights replicated):
  P0  every core computes the FULL v table (fp16) into a 5-stripe DRAM
      layout: 32767 real rows + 1 dummy(-60000) row per stripe, so each
      stripe is addressable by int16 dma_gather indices.
  P1  per 128-point tile (partition p = point p): multi-queue dma_gather
      pulls all neighbor rows (window-sorted + padded per a host-built
      static schedule; dummies land on the -60000 row);
      in-place DVE max(x, ctr)  [max(x,c) = relu(x-c)+c, dummy -> c];
      j-sum via identity-matmul PSUM accumulation over B(t) blocks;
      lap = psum/16 - (B(t)/16)*ctr        (B(t) is compile-time);
      TensorE transpose; hT = W2.T @ lapT  -> channels on partitions;
      per-channel BN partial sums; hT stored to DRAM (fp16).
  P2  AllReduce the [128,2] BN stats over the 8 cores; scale/shift.
  P3  hT chunks: scale*h+shift, relu, DMA out (u_tt.T per core).

Host: per-core point permutation groups points with similar per-window
neighbor-count profiles into tiles (gather padding ~2.2x -> ~1.3x); the
call schedule is the max over cores so the SPMD graph is identical.
"""

import sys

sys.path.insert(0, "/opt/trn_rl_repo")

import numpy as np

import concourse.bass as bass
import concourse.mybir as mybir
import concourse.tile as tile
from concourse import bacc

F32 = mybir.dt.float32
FP16 = mybir.dt.float16
I16 = mybir.dt.int16

NCORES = 8
C = 128
NS = 16
BN_EPS = 1e-5
DUMMY_VAL = -60000.0
MAXBLK = 8  # dma_gather num_idxs <= 1024  ->  <= 8 blocks of 128 rows


def _dims(N=131072, STRIPE=32767):
    d = {}
    d["N"] = N
    d["STRIPE"] = STRIPE
    d["NSTRIPES"] = (N + STRIPE - 1) // STRIPE
    d["SROWS"] = STRIPE + 1
    d["SH"] = N // NCORES
    d["TILES"] = d["SH"] // 128
    d["CH"] = 8  # P0 point-tiles per chunk
    return d


DIMS = _dims()


def _host_prepare(d, u, idx, W1, W2, gamma, beta):
    N, SH, TILES = d["N"], d["SH"], d["TILES"]
    STRIPE, NSTRIPES = d["STRIPE"], d["NSTRIPES"]
    win = (idx // STRIPE).astype(np.int64)
    loc = (idx - win * STRIPE).astype(np.int64)

    perms, counts = [], []
    for k in range(NCORES):
        w_k = win[k * SH : (k + 1) * SH]
        prof = np.stack([(w_k == q).sum(1) for q in range(NSTRIPES)], 1)
        key = np.zeros(SH, np.int64)
        for q in range(NSTRIPES - 1):
            key = key * (NS + 1) + prof[:, q]
        perm = np.argsort(key, kind="stable")
        # local refinement: snake order on the secondary axis reduces the
        # within-tile spread of later windows
        perms.append(perm)
        counts.append(prof[perm])

    B = np.zeros((TILES, NSTRIPES), np.int64)
    for k in range(NCORES):
        c = counts[k].reshape(TILES, 128, NSTRIPES)
        B = np.maximum(B, c.max(axis=1))

    callspec = []
    for t in range(TILES):
        calls, off = [], 0
        for w in range(NSTRIPES):
            rem = int(B[t, w])
            while rem > 0:
                nb = min(rem, MAXBLK)
                calls.append((w, nb, off))
                off += nb
                rem -= nb
        callspec.append((calls, off))

    idxcols = sum(8 * nb for calls, _ in callspec for (_, nb, _) in calls)

    idx_inputs = []
    for k in range(NCORES):
        perm = perms[k]
        w_k = win[k * SH : (k + 1) * SH][perm]
        l_k = loc[k * SH : (k + 1) * SH][perm]
        arr = np.full((128, idxcols), STRIPE, np.int16)  # default dummy
        col = 0
        for t in range(TILES):
            calls, _ = callspec[t]
            wt = w_k[t * 128 : (t + 1) * 128]
            lt = l_k[t * 128 : (t + 1) * 128]
            per_w = [
                [lt[p][wt[p] == w] for w in range(NSTRIPES)] for p in range(128)
            ]
            consumed = [[0] * NSTRIPES for _ in range(128)]
            for (w, nb, _off) in calls:
                ncols = 8 * nb
                sub = np.full((16, ncols), STRIPE, np.int16)
                for p in range(128):
                    lst = per_w[p][w]
                    c0 = consumed[p][w]
                    take = lst[c0 : c0 + nb]
                    consumed[p][w] = c0 + len(take)
                    for j, v in enumerate(take):
                        k_ord = j * 128 + p
                        sub[k_ord % 16, k_ord // 16] = v
                arr[0:16, col : col + ncols] = sub
                col += ncols
        for g in range(1, 8):
            arr[g * 16 : (g + 1) * 16] = arr[0:16]
        idx_inputs.append(arr)

    uT16 = np.ascontiguousarray(u.T.astype(np.float16))
    in_maps = []
    for k in range(NCORES):
        in_maps.append(
            {
                "uT": uT16,
                "uTo": np.ascontiguousarray(
                    u[k * SH : (k + 1) * SH][perms[k]].T.astype(np.float16)
                ),
                "idxg": idx_inputs[k],
                "W1": W1.astype(np.float16),
                "W2": W2.astype(np.float16),
                "ident": (np.eye(128) / 16.0).astype(np.float16),
                "gamma": gamma.astype(np.float32).reshape(C, 1),
                "beta": beta.astype(np.float32).reshape(C, 1),
            }
        )
    return in_maps, perms, callspec, idxcols


def _build(d, callspec, idxcols):
    N, SH, TILES, CH = d["N"], d["SH"], d["TILES"], d["CH"]
    STRIPE, NSTRIPES, SROWS = d["STRIPE"], d["NSTRIPES"], d["SROWS"]
    DUMMY_LOCAL = STRIPE

    nc = bacc.Bacc(target_bir_lowering=False, debug=False, num_swdge_queues=4)

    uT_d = nc.declare_dram_parameter("uT", [C, N], FP16, isOutput=False)
    uTo_d = nc.declare_dram_parameter("uTo", [C, SH], FP16, isOutput=False)
    idx_d = nc.declare_dram_parameter("idxg", [128, idxcols], I16, isOutput=False)
    W1_d = nc.declare_dram_parameter("W1", [C, C], FP16, isOutput=False)
    W2_d = nc.declare_dram_parameter("W2", [C, C], FP16, isOutput=False)
    id_d = nc.declare_dram_parameter("ident", [128, 128], FP16, isOutput=False)
    ga_d = nc.declare_dram_parameter("gamma", [C, 1], F32, isOutput=False)
    be_d = nc.declare_dram_parameter("beta", [C, 1], F32, isOutput=False)
    out_d = nc.declare_dram_parameter("out", [C, SH], F32, isOutput=True)

    v5 = nc.dram_tensor("v5", [NSTRIPES * SROWS, C], FP16)
    h_dram = nc.dram_tensor("h_dram", [C, SH], FP16)
    ccin = nc.dram_tensor("ccin", [128, 2], F32)
    ccout = nc.dram_tensor("ccout", [128, 2], F32, addr_space="Shared")

    Bmax = max(off for _, off in callspec)
    wbmax = {}
    for calls, _ in callspec:
        per_w = {}
        for (w, nb, _o) in calls:
            per_w[w] = per_w.get(w, 0) + nb
        for w, b in per_w.items():
            wbmax[w] = max(wbmax.get(w, 0), b)

    boundaries = [s * STRIPE for s in range(1, NSTRIPES)]

    with tile.TileContext(nc) as tc:
        with (
            tc.tile_pool(name="const", bufs=1) as cp,
            tc.tile_pool(name="ld", bufs=3) as lp,
            tc.tile_pool(name="work", bufs=2) as wp,
            tc.tile_pool(name="big", bufs=2) as bp,
            tc.tile_pool(name="p0", bufs=3) as p0p,
            tc.tile_pool(name="ps", bufs=2, space="PSUM") as pp,
        ):
            W1 = cp.tile([C, C], FP16, tag="W1")
            nc.sync.dma_start(out=W1[:], in_=W1_d[:, :])
            W2 = cp.tile([C, C], FP16, tag="W2")
            nc.sync.dma_start(out=W2[:], in_=W2_d[:, :])
            ident = cp.tile([128, 128], FP16, tag="ident")
            nc.sync.dma_start(out=ident[:], in_=id_d[:, :])
            stats = cp.tile([128, 2 * TILES], F32, tag="stats")
            bn = cp.tile([128, 16], F32, tag="bn")
            nc.sync.dma_start(out=bn[:, 8:9], in_=ga_d[:, :])
            nc.sync.dma_start(out=bn[:, 9:10], in_=be_d[:, :])

            idxg = cp.tile([128, idxcols], I16, tag="idxg")
            nc.sync.dma_start(out=idxg[:], in_=idx_d[:, :])
            dummy = cp.tile([1, C], FP16, tag="dummy")
            nc.vector.memset(dummy[:], DUMMY_VAL)
            for s in range(NSTRIPES):
                nc.sync.dma_start(
                    out=v5[s * SROWS + DUMMY_LOCAL : s * SROWS + DUMMY_LOCAL + 1, :],
                    in_=dummy[:],
                )

            # ---------------- P0: v table ----------------
            def emit_rows(vt, g0, nrows):
                """DMA vt ([128, CH*C], tile i col-block = rows g0+128i..) to v5."""
                g1 = g0 + nrows
                straddle = any(g0 < b < g1 for b in boundaries)
                if not straddle:
                    s = g0 // STRIPE
                    dst = s * SROWS + (g0 - s * STRIPE)
                    nc.sync.dma_start(
                        out=v5[dst : dst + nrows, :].rearrange(
                            "(i p) c -> p i c", p=128
                        ),
                        in_=vt[:, : (nrows // 128) * C].rearrange(
                            "p (i c) -> p i c", c=C
                        ),
                    )
                    return
                for i in range(nrows // 128):
                    r0 = g0 + i * 128
                    bs = [b for b in boundaries if r0 < b < r0 + 128]
                    cuts = [r0] + bs + [r0 + 128]
                    for a, b_ in zip(cuts[:-1], cuts[1:]):
                        s = a // STRIPE
                        dst = s * SROWS + (a - s * STRIPE)
                        nc.sync.dma_start(
                            out=v5[dst : dst + (b_ - a), :],
                            in_=vt[a - r0 : b_ - r0, i * C : (i + 1) * C],
                        )

            for ch in range(N // (128 * CH)):
                ut = p0p.tile([C, 128 * CH], FP16, tag="ut")
                nc.sync.dma_start(
                    out=ut[:], in_=uT_d[:, ch * 128 * CH : (ch + 1) * 128 * CH]
                )
                vt = p0p.tile([128, CH * C], FP16, tag="vt")
                for i in range(CH):
                    ps = pp.tile([128, C], F32, tag="acc", space="PSUM")
                    nc.tensor.matmul(
                        out=ps[:],
                        lhsT=ut[:, i * 128 : (i + 1) * 128],
                        rhs=W1[:],
                        start=True,
                        stop=True,
                    )
                    if i % 2 == 0:
                        nc.scalar.mul(vt[:, i * C : (i + 1) * C], ps[:], 1.0)
                    else:
                        nc.vector.tensor_copy(
                            out=vt[:, i * C : (i + 1) * C], in_=ps[:]
                        )
                emit_rows(vt, ch * 128 * CH, 128 * CH)

            # ---------------- P1 ----------------
            # pre-loaded num_idxs registers (one per distinct value) so the
            # gather calls carry no per-call register MOVE (a MOVE to a
            # register read by an in-flight gather serializes the queue)
            nb_vals = sorted({nb for calls, _ in callspec for (_, nb, _) in calls})
            nidx_regs = {nb: nc.gpsimd.to_reg(128 * nb) for nb in nb_vals}
            gsem = nc.alloc_semaphore("gsem")
            cum_calls = [0]
            for calls, _ in callspec:
                cum_calls.append(cum_calls[-1] + len(calls))
            qctr = 0
            col = 0
            for t in range(TILES):
                calls, Bt = callspec[t]
                ncols_t = sum(8 * nb for (_, nb, _) in calls)

                uot = lp.tile([C, 128], FP16, tag="uot")
                nc.sync.dma_start(
                    out=uot[:], in_=uTo_d[:, t * 128 : (t + 1) * 128]
                )

                psc = pp.tile([128, C], F32, tag="psc", space="PSUM")
                nc.tensor.matmul(
                    out=psc[:], lhsT=uot[:], rhs=W1[:], start=True, stop=True
                )
                ctr = wp.tile([128, C], FP16, tag="ctr")
                nc.scalar.mul(ctr[:], psc[:], 1.0)

                gtiles = {}
                mtiles = {}
                for (w, nb, boff) in calls:
                    if w not in gtiles:
                        gtiles[w] = (
                            bp.tile(
                                [128, wbmax[w] * C], FP16, tag=f"g{w}",
                                name=f"g{w}_t{t}",
                            ),
                            bp.tile(
                                [128, wbmax[w] * C], FP16, tag=f"m{w}",
                                name=f"m{w}_t{t}",
                            ),
                        )
                ccol = 0
                with tc.tile_critical():
                    if t >= 2:
                        nc.gpsimd.wait_ge(gsem, 16 * cum_calls[t - 1])
                    for (w, nb, boff) in calls:
                        gw = gtiles[w][0]
                        woff = sum(
                            b2 for (w2, b2, o2) in calls if w2 == w and o2 < boff
                        )
                        nc.gpsimd.dma_gather(
                            gw[:, woff * C : (woff + nb) * C].rearrange(
                                "p (b e) -> p b e", e=C
                            ),
                            v5[w * SROWS : (w + 1) * SROWS, :],
                            idxg[:, col + ccol : col + ccol + 8 * nb],
                            128 * nb,
                            nidx_regs[nb],
                            C,
                            queue_num=qctr % 4,
                        ).then_inc(gsem, 16)
                        qctr += 1
                        ccol += 8 * nb
                col += ncols_t

                gate = 16 * cum_calls[t + 1]
                for w, (gw, mw) in gtiles.items():
                    bw = wblocks[w]
                    with tc.tile_critical():
                        nc.vector.tensor_tensor(
                            out=mw[:, : bw * C].rearrange("p (b c) -> p b c", c=C),
                            in0=gw[:, : bw * C].rearrange("p (b c) -> p b c", c=C),
                            in1=ctr[:]
                            .rearrange("p (o c) -> p o c", o=1)
                            .to_broadcast([128, bw, C]),
                            op=mybir.AluOpType.max,
                        )._wait_ge(gsem, gate)

                psl = pp.tile([128, C], F32, tag="acc", space="PSUM")
                nmm = sum(wblocks.values())
                bi = 0
                for w in sorted(gtiles):
                    mw = gtiles[w][1]
                    for b in range(wblocks[w]):
                        nc.tensor.matmul(
                            out=psl[:],
                            lhsT=ident[:],
                            rhs=mw[:, b * C : (b + 1) * C],
                            start=(bi == 0),
                            stop=(bi == nmm - 1),
                        )
                        bi += 1

                ctrs = wp.tile([128, C], FP16, tag="ctrs")
                nc.scalar.mul(ctrs[:], psc[:], float(Bt) / 16.0)
                lap = wp.tile([128, C], FP16, tag="lap")
                nc.vector.tensor_tensor(
                    out=lap[:], in0=psl[:], in1=ctrs[:],
                    op=mybir.AluOpType.subtract,
                )

                pst = pp.tile([128, C], FP16, tag="pst", space="PSUM")
                nc.tensor.transpose(out=pst[:], in_=lap[:], identity=ident[:])
                lapT = wp.tile([128, C], FP16, tag="lapT")
                nc.scalar.mul(lapT[:], pst[:], 1.0)

                psh = pp.tile([128, C], F32, tag="psh", space="PSUM")
                nc.tensor.matmul(
                    out=psh[:], lhsT=W2[:], rhs=lapT[:], start=True, stop=True
                )
                h16 = wp.tile([128, C], FP16, tag="h16")
                nc.scalar.activation(
                    h16[:],
                    psh[:],
                    mybir.ActivationFunctionType.Copy,
                    accum_out=stats[:, t : t + 1],
                )
                nc.sync.dma_start(
                    out=h_dram[:, t * 128 : (t + 1) * 128], in_=h16[:]
                )
                sqd = wp.tile([128, C], FP16, tag="sqd")
                nc.scalar.activation(
                    sqd[:],
                    psh[:],
                    mybir.ActivationFunctionType.Square,
                    accum_out=stats[:, TILES + t : TILES + t + 1],
                )

            # ---------------- P2: BN stats ----------------
            nc.vector.reduce_sum(
                out=bn[:, 0:1], in_=stats[:, :TILES], axis=mybir.AxisListType.X
            )
            nc.vector.reduce_sum(
                out=bn[:, 1:2], in_=stats[:, TILES:], axis=mybir.AxisListType.X
            )
            cc_sem = nc.alloc_semaphore("cc_sem")
            dma_sem = nc.alloc_semaphore("cc_dma")
            with tc.tile_critical():
                nc.gpsimd.dma_start(out=ccin[:, :], in_=bn[:, 0:2]).then_inc(
                    dma_sem, 16
                )
                nc.gpsimd.wait_ge(dma_sem, 16)
                nc.gpsimd.collective_compute(
                    "AllReduce",
                    mybir.AluOpType.add,
                    replica_groups=[list(range(NCORES))],
                    ins=[ccin[:, :]],
                    outs=[ccout[:, :]],
                ).then_inc(cc_sem, 1)
                nc.gpsimd.wait_ge(cc_sem, 1)
                nc.gpsimd.dma_start(out=bn[:, 2:4], in_=ccout[:, :]).then_inc(
                    dma_sem, 16
                )
                nc.gpsimd.wait_ge(dma_sem, 32)

            nc.vector.tensor_scalar(
                out=bn[:, 4:5], in0=bn[:, 2:3], scalar1=1.0 / N, scalar2=None,
                op0=mybir.AluOpType.mult,
            )  # mean
            nc.vector.tensor_scalar(
                out=bn[:, 5:6], in0=bn[:, 3:4], scalar1=1.0 / N, scalar2=None,
                op0=mybir.AluOpType.mult,
            )  # E[x^2]
            nc.vector.tensor_tensor(
                out=bn[:, 6:7], in0=bn[:, 4:5], in1=bn[:, 4:5],
                op=mybir.AluOpType.mult,
            )
            nc.vector.tensor_tensor(
                out=bn[:, 7:8], in0=bn[:, 5:6], in1=bn[:, 6:7],
                op=mybir.AluOpType.subtract,
            )  # var
            nc.vector.tensor_scalar(
                out=bn[:, 10:11], in0=bn[:, 7:8], scalar1=BN_EPS, scalar2=None,
                op0=mybir.AluOpType.add,
            )
            nc.vector.reciprocal(out=bn[:, 15:16], in_=bn[:, 10:11])
            nc.scalar.activation(
                bn[:, 11:12], bn[:, 15:16], mybir.ActivationFunctionType.Sqrt
            )
            nc.vector.tensor_tensor(
                out=bn[:, 12:13], in0=bn[:, 8:9], in1=bn[:, 11:12],
                op=mybir.AluOpType.mult,
            )  # scale
            nc.vector.tensor_tensor(
                out=bn[:, 13:14], in0=bn[:, 4:5], in1=bn[:, 12:13],
                op=mybir.AluOpType.mult,
            )
            nc.vector.tensor_tensor(
                out=bn[:, 14:15], in0=bn[:, 9:10], in1=bn[:, 13:14],
                op=mybir.AluOpType.subtract,
            )  # shift

            # ---------------- P3 ----------------
            PC = 8
            for ch in range(TILES // PC):
                hch = wp.tile([128, 128 * PC], FP16, tag="hch")
                nc.sync.dma_start(
                    out=hch[:],
                    in_=h_dram[:, ch * 128 * PC : (ch + 1) * 128 * PC],
                )
                o2 = wp.tile([128, 128 * PC], F32, tag="o2")
                nc.scalar.activation(
                    o2[:],
                    hch[:],
                    mybir.ActivationFunctionType.Relu,
                    bias=bn[:, 14:15],
                    scale=bn[:, 12:13],
                )
                nc.sync.dma_start(
                    out=out_d[:, ch * 128 * PC : (ch + 1) * 128 * PC], in_=o2[:]
                )

    nc.compile()
    return nc


def _run(d, u, idx, W1, W2, gamma, beta, run_kwargs=None):
    from concourse.bass_utils import run_bass_kernel_spmd

    in_maps, perms, callspec, idxcols = _host_prepare(
        d, u, idx, W1, W2, gamma, beta
    )
    nc = _build(d, callspec, idxcols)
    res = run_bass_kernel_spmd(
        nc, in_maps, core_ids=list(range(NCORES)), **(run_kwargs or {})
    )
    SH, N = d["SH"], d["N"]
    u_tt = np.empty((N, C), np.float32)
    for k in range(NCORES):
        shard = res.results[k]["out"].T
        inv = np.empty(SH, np.int64)
        inv[perms[k]] = np.arange(SH)
        u_tt[k * SH : (k + 1) * SH] = shard[inv]
    return u_tt, res


def kernel(p, u, o, idx, W1, b1, W2, b2, gamma, beta):
    u_tt, _ = _run(
        DIMS,
        np.asarray(u, np.float32),
        np.asarray(idx, np.int64),
        np.asarray(W1),
        np.asarray(W2),
        np.asarray(gamma),
        np.asarray(beta),
    )
    return (
        np.asarray(p, np.float32),
        u_tt,
        np.asarray(o, np.int32),
    )
